# revision 18
# baseline (speedup 1.0000x reference)
"""Bahdanau-attention kernel for Trainium2, data-parallel over batch on 8 cores.

Computation (per batch b):
    q   = x[b] @ Wq.T + bq + bm                    # [H]      (stage A)
    pre = ctx[b] @ Wm.T (+ q broadcast)            # [T, H]   (main matmuls)
    s   = tanh(pre) . v                            # [T]      (v-dot matmuls)
    e   = exp(s);  L = sum(e)                      # softmax without max-sub
    score = e / L
    c   = sum_t e[t] * ctx[b,t,:] / L              # weighted ctx sum (DVE)

Layout: all device inputs are pre-arranged on the host into the SBUF tile
layout [partition, h-chunk, ...] (fp16 on the matmul path) so every DMA is
contiguous per partition with fat descriptors. Context is transposed to
[B, p, c, T] and stays SBUF-resident for a whole batch. The main matmul
contracts H on the partition dim; exp(s) is broadcast across partitions via
a ones-matmul and the weighted context sum runs on the vector engine.
"""

import numpy as np

B, T, H = 32, 2048, 1024
NCORES = 8
BL = B // NCORES  # batches per core
HC = H // 128     # 128-partition chunks of H
TT = 512          # t-tile (one fp32 PSUM bank)
NTT = T // TT

_CACHE = {}


def _build():
    import concourse.bass as bass
    import concourse.mybir as mybir
    import concourse.tile as tile

    F32 = mybir.dt.float32
    F16 = mybir.dt.float16
    AF = mybir.ActivationFunctionType
    ALU = mybir.AluOpType

    nc = bass.Bass("TRN2", target_bir_lowering=False, debug=False)

    ctxh = nc.dram_tensor("ctxh", [BL, 128, HC, T], F16, kind="ExternalInput").ap()
    wmh = nc.dram_tensor("wmh", [128, HC, H], F16, kind="ExternalInput").ap()
    wqh = nc.dram_tensor("wqh", [128, HC, H], F16, kind="ExternalInput").ap()
    xh = nc.dram_tensor("xh", [128, HC, BL], F16, kind="ExternalInput").ap()
    vh = nc.dram_tensor("vh", [128, HC], F16, kind="ExternalInput").ap()
    bqh = nc.dram_tensor("bqh", [128, HC], F32, kind="ExternalInput").ap()
    bmh = nc.dram_tensor("bmh", [128, HC], F32, kind="ExternalInput").ap()
    c_out = nc.dram_tensor("c_out", [BL, H], F32, kind="ExternalOutput").ap()
    p_out = nc.dram_tensor("p_out", [BL, T], F32, kind="ExternalOutput").ap()

    with tile.TileContext(nc) as tc:
        with (
            tc.tile_pool(name="weights", bufs=1) as wpool,
            tc.tile_pool(name="stream", bufs=3) as spool,
            tc.tile_pool(name="small", bufs=2) as mpool,
            tc.tile_pool(name="psum", bufs=1, space="PSUM") as ppool,
        ):
            # ---- PE warm-up: dependency-free full-K matmuls flip the HAM
            # clock gate to 8/8 while the prologue DMAs are in flight
            junkA_sb = wpool.tile([128, 128], F16)
            nc.vector.memset(junkA_sb, 0.0)
            junkB_sb = wpool.tile([128, TT], F16)
            nc.vector.memset(junkB_sb, 0.0)
            warm_ps = ppool.tile([128, TT], F32, tag="wps", bufs=2, name="warm_ps")
            ones_sb = wpool.tile([1, 128], F16)
            nc.vector.memset(ones_sb, 1.0)
            for _ in range(20):
                nc.tensor.matmul(
                    warm_ps, lhsT=junkA_sb, rhs=junkB_sb, start=True, stop=True
                )

            # ---- resident tensors -------------------------------------
            # batch-0 context chunks on the SWDGE path (they gate the first
            # matmuls); Wm halves take the two HWDGE queues
            ctx00_sb = spool.tile(
                [128, HC, TT], F16, tag="ctx", bufs=3, name="ctx00_sb"
            )
            for c in range(HC):
                nc.gpsimd.dma_start(
                    out=ctx00_sb[:, c, :], in_=ctxh[0, :, c, 0:TT]
                )
            wm_sb = wpool.tile([128, HC, H], F16)  # [p, hchunk, o]
            nc.sync.dma_start(out=wm_sb[:, 0:2, :], in_=wmh[:, 0:2, :])
            nc.scalar.dma_start(out=wm_sb[:, 2:4, :], in_=wmh[:, 2:4, :])
            nc.sync.dma_start(out=wm_sb[:, 4:6, :], in_=wmh[:, 4:6, :])
            nc.scalar.dma_start(out=wm_sb[:, 6:8, :], in_=wmh[:, 6:8, :])
            xT_sb = wpool.tile([128, HC, BL], F16)
            nc.gpsimd.dma_start(out=xT_sb, in_=xh)
            v_sb = wpool.tile([128, HC], F16)
            nc.gpsimd.dma_start(out=v_sb, in_=vh)
            bq_sb = wpool.tile([128, HC], F32)
            nc.scalar.dma_start(out=bq_sb, in_=bqh)
            bm_sb = wpool.tile([128, HC], F32)
            nc.scalar.dma_start(out=bm_sb, in_=bmh)
            bqm_sb = wpool.tile([128, HC], F32)
            nc.vector.tensor_add(bqm_sb, bq_sb, bm_sb)
            wq_sb = wpool.tile([128, HC, H], F16)
            nc.sync.dma_start(out=wq_sb[:, 0:4, :], in_=wqh[:, 0:4, :])
            nc.scalar.dma_start(out=wq_sb[:, 4:8, :], in_=wqh[:, 4:8, :])
            ones32_sb = wpool.tile([1, 128], F32)
            nc.vector.memset(ones32_sb, 1.0)
            qb_sb = wpool.tile([128, HC, BL], F32)  # q + bq + bm, [o-part, ot, b]

            # ---- main loop --------------------------------------------
            for b in range(BL):
                e_sb = mpool.tile([1, T], F16, tag="e", bufs=2)
                lpart = mpool.tile([1, NTT], F32, tag="lp", bufs=2)
                u_sb = mpool.tile([128, HC], F32, tag="u", bufs=2)
                nc.vector.memset(u_sb, 0.0)
                for tt in range(NTT):
                    tsl = slice(tt * TT, (tt + 1) * TT)
                    if b == 0 and tt == 0:
                        ctx_t = ctx00_sb
                    else:
                        ctx_t = spool.tile([128, HC, TT], F16, tag="ctx", bufs=3)
                        eng = [nc.sync, nc.scalar, nc.gpsimd][(b * NTT + tt) % 3]
                        eng.dma_start(out=ctx_t, in_=ctxh[b, :, :, tsl])
                    s_ps = ppool.tile([1, TT], F32, tag="sps", bufs=3)

                    def tile_epilogue(ot, hc_ps):
                        if b == 0 and tt == 0:
                            # stage A interleaved with the first tile's mains
                            q_ps = ppool.tile(
                                [128, BL], F32, tag="sps", bufs=3, name="q_ps"
                            )
                            for c in range(HC):
                                nc.tensor.matmul(
                                    q_ps,
                                    lhsT=wq_sb[:, c, ot * 128:(ot + 1) * 128],
                                    rhs=xT_sb[:, c, :],
                                    start=(c == 0),
                                    stop=(c == HC - 1),
                                )
                            nc.scalar.activation(
                                qb_sb[:, ot, :], q_ps, AF.Identity,
                                bias=bqm_sb[:, ot:ot + 1], scale=1.0,
                            )
                        th_sb = spool.tile([128, TT], F16, tag="tanh", bufs=3)
                        nc.scalar.activation(
                            th_sb, hc_ps, AF.Tanh,
                            bias=qb_sb[:, ot, b:b + 1], scale=1.0,
                        )
                        nc.tensor.matmul(
                            s_ps,
                            lhsT=v_sb[:, ot:ot + 1],
                            rhs=th_sb,
                            start=(ot == 0),
                            stop=(ot == HC - 1),
                        )

                    if b == 0 and tt == 0:
                        # c-outer in groups of <=3 o-tiles so the first mains
                        # consume Wm/ctx chunks at their DMA arrival rate
                        for grp in ((0, 1, 2), (3, 4, 5), (6, 7)):
                            tiles = {
                                ot: ppool.tile(
                                    [128, TT], F32, tag="hcps", bufs=3,
                                    name=f"hc_ps_g{ot}",
                                )
                                for ot in grp
                            }
                            for c in range(HC):
                                for ot in grp:
                                    nc.tensor.matmul(
                                        tiles[ot],
                                        lhsT=wm_sb[:, c, ot * 128:(ot + 1) * 128],
                                        rhs=ctx_t[:, c, :],
                                        start=(c == 0),
                                        stop=(c == HC - 1),
                                    )
                            for ot in grp:
                                tile_epilogue(ot, tiles[ot])
                    else:
                        for ot in range(HC):
                            hc_ps = ppool.tile([128, TT], F32, tag="hcps", bufs=3)
                            for c in range(HC):
                                nc.tensor.matmul(
                                    hc_ps,
                                    lhsT=wm_sb[:, c, ot * 128:(ot + 1) * 128],
                                    rhs=ctx_t[:, c, :],
                                    start=(c == 0),
                                    stop=(c == HC - 1),
                                )
                            tile_epilogue(ot, hc_ps)

                    e_slice = e_sb[:, tsl]
                    nc.scalar.activation(
                        e_slice, s_ps, AF.Exp,
                        scale=1.0, accum_out=lpart[:, tt:tt + 1],
                    )
                    w_ps = ppool.tile([128, TT], F32, tag="wps", bufs=2)
                    nc.tensor.matmul(
                        w_ps, lhsT=ones_sb, rhs=e_slice, start=True, stop=True
                    )
                    w16_sb = spool.tile([128, TT], F16, tag="w16", bufs=2)
                    nc.vector.tensor_copy(w16_sb, w_ps)
                    scr_sb = spool.tile([128, HC, TT], F16, tag="scr", bufs=2)
                    red_sb = mpool.tile([128, HC], F32, tag="red", bufs=2)
                    for c in range(HC):
                        nc.vector.tensor_mul(
                            scr_sb[:, c, :], ctx_t[:, c, :], w16_sb
                        )
                        nc.vector.tensor_reduce(
                            red_sb[:, c:c + 1], scr_sb[:, c, :],
                            axis=mybir.AxisListType.X, op=ALU.add,
                        )
                    nc.vector.tensor_add(u_sb, u_sb, red_sb)
                # normalize + write outputs
                l_sum = mpool.tile([1, 1], F32, tag="lsum", bufs=2)
                nc.vector.tensor_reduce(
                    l_sum, lpart, axis=mybir.AxisListType.X, op=ALU.add
                )
                rcp = mpool.tile([1, 1], F32, tag="rcp", bufs=2)
                nc.vector.reciprocal(rcp, l_sum)
                p_sb = mpool.tile([1, T], F32, tag="p", bufs=2)
                nc.scalar.activation(p_sb, e_sb, AF.Copy, bias=0.0, scale=rcp)
                nc.sync.dma_start(out=p_out[b:b + 1, :], in_=p_sb)
                rcpb_ps = ppool.tile([128, 1], F32, tag="wps", bufs=2)
                nc.tensor.matmul(
                    rcpb_ps, lhsT=ones32_sb, rhs=rcp, start=True, stop=True
                )
                rcpb_sb = mpool.tile([128, 1], F32, tag="rcpb", bufs=2)
                nc.scalar.copy(rcpb_sb, rcpb_ps)
                c_sb = mpool.tile([128, HC], F32, tag="c", bufs=2)
                nc.scalar.activation(c_sb, u_sb, AF.Copy, bias=0.0, scale=rcpb_sb)
                nc.sync.dma_start(
                    out=c_out[b].rearrange("(c p) -> p c", p=128), in_=c_sb
                )

    return nc


def _fix_matmul_waits(nc):
    """Walrus accepts a single sync-wait per instruction on this build; move
    extra waits onto same-engine EventSemaphore ops inserted just before."""
    import concourse.mybir as mybir

    for f in nc.m.functions:
        for bb in f.blocks:
            new = []
            for inst in bb.instructions:
                si = getattr(inst, "sync_info", None)
                if (
                    si is not None
                    and len(si.on_wait) > 1
                    and getattr(inst, "engine", None) is not None
                ):
                    waits = list(si.on_wait)
                    for w in waits[:-1]:
                        new.append(
                            mybir.InstEventSemaphore(
                                name=nc.get_next_instruction_name(),
                                engine=inst.engine,
                                sync_info=mybir.SyncInfo(on_wait=[w], on_update=[]),
                            )
                        )
                    inst.sync_info = mybir.SyncInfo(
                        on_wait=[waits[-1]], on_update=list(si.on_update)
                    )
                new.append(inst)
            bb.instructions[:] = new
    return nc


def _get_nc():
    if "nc" not in _CACHE:
        _CACHE["nc"] = _fix_matmul_waits(_build())
    return _CACHE["nc"]


def _prep_in_maps(x, context, Wq, bq, Wm, bm, v):
    x = np.asarray(x, dtype=np.float32)
    context = np.asarray(context, dtype=np.float32)

    def chunked(a):
        # [H, ...] -> [128, HC, ...] with h = c*128 + p
        return np.ascontiguousarray(
            a.reshape(HC, 128, *a.shape[1:]).transpose(1, 0, *range(2, a.ndim + 1))
        )

    wmh = chunked(np.asarray(Wm, dtype=np.float32).T.astype(np.float16))
    wqh = chunked(np.asarray(Wq, dtype=np.float32).T.astype(np.float16))
    vh = chunked(np.asarray(v, dtype=np.float32).astype(np.float16))
    bqh = chunked(np.asarray(bq, dtype=np.float32))
    bmh = chunked(np.asarray(bm, dtype=np.float32))

    in_maps = []
    for i in range(NCORES):
        sl = slice(i * BL, (i + 1) * BL)
        # context[b].T -> [H, T] -> [128, HC, T] per batch
        ctx = context[sl].transpose(0, 2, 1).astype(np.float16)
        ctxh = np.ascontiguousarray(ctx.reshape(BL, HC, 128, T).transpose(0, 2, 1, 3))
        xhp = chunked(x[sl].T.astype(np.float16))
        in_maps.append(
            {
                "ctxh": ctxh,
                "wmh": wmh,
                "wqh": wqh,
                "xh": xhp,
                "vh": vh,
                "bqh": bqh,
                "bmh": bmh,
            }
        )
    return in_maps


def kernel(x, context, mask, Wq, bq, Wm, bm, v, _trace=False):
    from concourse import bass_utils

    nc = _get_nc()
    in_maps = _prep_in_maps(x, context, Wq, bq, Wm, bm, v)
    res = bass_utils.run_bass_kernel_spmd(
        nc, in_maps, core_ids=list(range(NCORES)), trace=_trace
    )
    if _trace and res.exec_time_ns is not None:
        print(f"HW exec time: {res.exec_time_ns} ns")
    c = np.concatenate([res.results[i]["c_out"] for i in range(NCORES)], axis=0)
    p = np.concatenate([res.results[i]["p_out"] for i in range(NCORES)], axis=0)
    return c, p


# revision 19
# speedup vs baseline: 1.0079x; 1.0079x over previous
"""Bahdanau-attention kernel for Trainium2, data-parallel over batch on 8 cores.

Computation (per batch b):
    q   = x[b] @ Wq.T + bq + bm                    # [H]      (stage A)
    pre = ctx[b] @ Wm.T (+ q broadcast)            # [T, H]   (main matmuls)
    s   = tanh(pre) . v                            # [T]      (v-dot matmuls)
    e   = exp(s);  L = sum(e)                      # softmax without max-sub
    score = e / L
    c   = sum_t e[t] * ctx[b,t,:] / L              # weighted ctx sum (DVE)

Layout: all device inputs are pre-arranged on the host into the SBUF tile
layout [partition, h-chunk, ...] (fp16 on the matmul path) so every DMA is
contiguous per partition. Context streams in [128, HC, 512] tiles over three
DMA queues; the main matmul contracts H on the partition dim at 1 col/cycle
(fp16), exp(s) is broadcast across partitions via a ones-matmul, and the
weighted context sum runs on the vector engine. Dependency-free warm-up
matmuls flip the PE clock gate to full rate during the prologue DMAs.
"""

import numpy as np

B, T, H = 32, 2048, 1024
NCORES = 8
BL = B // NCORES  # batches per core
HC = H // 128     # 128-partition chunks of H
TT = 512          # t-tile (one fp32 PSUM bank)
NTT = T // TT

_CACHE = {}


def _build():
    import concourse.bass as bass
    import concourse.mybir as mybir
    import concourse.tile as tile

    F32 = mybir.dt.float32
    F16 = mybir.dt.float16
    AF = mybir.ActivationFunctionType
    ALU = mybir.AluOpType

    nc = bass.Bass("TRN2", target_bir_lowering=False, debug=False)

    ctxh = nc.dram_tensor("ctxh", [BL, 128, HC, T], F16, kind="ExternalInput").ap()
    wmh = nc.dram_tensor("wmh", [128, HC, H], F16, kind="ExternalInput").ap()
    wqh = nc.dram_tensor("wqh", [128, HC, H], F16, kind="ExternalInput").ap()
    xh = nc.dram_tensor("xh", [128, HC, BL], F16, kind="ExternalInput").ap()
    vh = nc.dram_tensor("vh", [128, HC], F16, kind="ExternalInput").ap()
    bqh = nc.dram_tensor("bqh", [128, HC], F32, kind="ExternalInput").ap()
    bmh = nc.dram_tensor("bmh", [128, HC], F32, kind="ExternalInput").ap()
    c_out = nc.dram_tensor("c_out", [BL, H], F32, kind="ExternalOutput").ap()
    p_out = nc.dram_tensor("p_out", [BL, T], F32, kind="ExternalOutput").ap()

    with tile.TileContext(nc) as tc:
        with (
            tc.tile_pool(name="weights", bufs=1) as wpool,
            tc.tile_pool(name="stream", bufs=3) as spool,
            tc.tile_pool(name="small", bufs=2) as mpool,
            tc.tile_pool(name="psum", bufs=1, space="PSUM") as ppool,
        ):
            # ---- PE warm-up: dependency-free full-K matmuls flip the HAM
            # clock gate to 8/8 while the prologue DMAs are in flight
            junkA_sb = wpool.tile([128, 128], F16)
            nc.vector.memset(junkA_sb, 0.0)
            junkB_sb = wpool.tile([128, TT], F16)
            nc.vector.memset(junkB_sb, 0.0)
            warm_ps = ppool.tile([128, TT], F32, tag="wps", bufs=2, name="warm_ps")
            ones_sb = wpool.tile([1, 128], F16)
            nc.vector.memset(ones_sb, 1.0)
            for _ in range(20):
                nc.tensor.matmul(
                    warm_ps, lhsT=junkA_sb, rhs=junkB_sb, start=True, stop=True
                )

            # ---- resident tensors -------------------------------------
            # batch-0 context chunks on the SWDGE path (they gate the first
            # matmuls); Wm halves take the two HWDGE queues
            ctx00_sb = spool.tile(
                [128, HC, TT], F16, tag="ctx", bufs=3, name="ctx00_sb"
            )
            for c in range(HC):
                nc.gpsimd.dma_start(
                    out=ctx00_sb[:, c, :], in_=ctxh[0, :, c, 0:TT]
                )
            wm_sb = wpool.tile([128, HC, H], F16)  # [p, hchunk, o]
            nc.sync.dma_start(out=wm_sb[:, 0:2, :], in_=wmh[:, 0:2, :])
            nc.scalar.dma_start(out=wm_sb[:, 2:4, :], in_=wmh[:, 2:4, :])
            nc.sync.dma_start(out=wm_sb[:, 4:6, :], in_=wmh[:, 4:6, :])
            nc.scalar.dma_start(out=wm_sb[:, 6:8, :], in_=wmh[:, 6:8, :])
            xT_sb = wpool.tile([128, HC, BL], F16)
            nc.gpsimd.dma_start(out=xT_sb, in_=xh)
            v_sb = wpool.tile([128, HC], F16)
            nc.gpsimd.dma_start(out=v_sb, in_=vh)
            bq_sb = wpool.tile([128, HC], F32)
            nc.scalar.dma_start(out=bq_sb, in_=bqh)
            bm_sb = wpool.tile([128, HC], F32)
            nc.scalar.dma_start(out=bm_sb, in_=bmh)
            bqm_sb = wpool.tile([128, HC], F32)
            nc.vector.tensor_add(bqm_sb, bq_sb, bm_sb)
            wq_sb = wpool.tile([128, HC, H], F16)
            nc.sync.dma_start(out=wq_sb[:, 0:4, :], in_=wqh[:, 0:4, :])
            nc.scalar.dma_start(out=wq_sb[:, 4:8, :], in_=wqh[:, 4:8, :])
            ones32_sb = wpool.tile([1, 128], F32)
            nc.vector.memset(ones32_sb, 1.0)
            qb_sb = wpool.tile([128, HC, BL], F32)  # q + bq + bm, [o-part, ot, b]

            # ---- main loop --------------------------------------------
            for b in range(BL):
                e_sb = mpool.tile([1, T], F16, tag="e", bufs=2)
                lpart = mpool.tile([1, NTT], F32, tag="lp", bufs=2)
                u_sb = mpool.tile([128, HC], F32, tag="u", bufs=2)
                nc.vector.memset(u_sb, 0.0)
                for tt in range(NTT):
                    tsl = slice(tt * TT, (tt + 1) * TT)
                    if b == 0 and tt == 0:
                        ctx_t = ctx00_sb
                    else:
                        ctx_t = spool.tile([128, HC, TT], F16, tag="ctx", bufs=3)
                        eng = [nc.sync, nc.scalar, nc.gpsimd][(b * NTT + tt) % 3]
                        eng.dma_start(out=ctx_t, in_=ctxh[b, :, :, tsl])
                    s_ps = ppool.tile([1, TT], F32, tag="sps", bufs=3)

                    def tile_epilogue(ot, hc_ps):
                        if b == 0 and tt == 0:
                            # stage A interleaved with the first tile's mains
                            q_ps = ppool.tile(
                                [128, BL], F32, tag="sps", bufs=3, name="q_ps"
                            )
                            for c in range(HC):
                                nc.tensor.matmul(
                                    q_ps,
                                    lhsT=wq_sb[:, c, ot * 128:(ot + 1) * 128],
                                    rhs=xT_sb[:, c, :],
                                    start=(c == 0),
                                    stop=(c == HC - 1),
                                )
                            nc.scalar.activation(
                                qb_sb[:, ot, :], q_ps, AF.Identity,
                                bias=bqm_sb[:, ot:ot + 1], scale=1.0,
                            )
                        th_sb = spool.tile([128, TT], F16, tag="tanh", bufs=3)
                        nc.scalar.activation(
                            th_sb, hc_ps, AF.Tanh,
                            bias=qb_sb[:, ot, b:b + 1], scale=1.0,
                        )
                        nc.tensor.matmul(
                            s_ps,
                            lhsT=v_sb[:, ot:ot + 1],
                            rhs=th_sb,
                            start=(ot == 0),
                            stop=(ot == HC - 1),
                        )

                    if b == 0 and tt == 0:
                        # c-outer in groups of <=3 o-tiles so the first mains
                        # consume Wm/ctx chunks at their DMA arrival rate
                        for grp in ((0, 1, 2), (3, 4, 5), (6, 7)):
                            tiles = {
                                ot: ppool.tile(
                                    [128, TT], F32, tag="hcps", bufs=3,
                                    name=f"hc_ps_g{ot}",
                                )
                                for ot in grp
                            }
                            for c in range(HC):
                                for ot in grp:
                                    nc.tensor.matmul(
                                        tiles[ot],
                                        lhsT=wm_sb[:, c, ot * 128:(ot + 1) * 128],
                                        rhs=ctx_t[:, c, :],
                                        start=(c == 0),
                                        stop=(c == HC - 1),
                                    )
                            for ot in grp:
                                tile_epilogue(ot, tiles[ot])
                    else:
                        for ot in range(HC):
                            hc_ps = ppool.tile([128, TT], F32, tag="hcps", bufs=3)
                            for c in range(HC):
                                nc.tensor.matmul(
                                    hc_ps,
                                    lhsT=wm_sb[:, c, ot * 128:(ot + 1) * 128],
                                    rhs=ctx_t[:, c, :],
                                    start=(c == 0),
                                    stop=(c == HC - 1),
                                )
                            tile_epilogue(ot, hc_ps)

                    e_slice = e_sb[:, tsl]
                    nc.scalar.activation(
                        e_slice, s_ps, AF.Exp,
                        scale=1.0, accum_out=lpart[:, tt:tt + 1],
                    )
                    w_ps = ppool.tile([128, TT], F32, tag="wps", bufs=2)
                    nc.tensor.matmul(
                        w_ps, lhsT=ones_sb, rhs=e_slice, start=True, stop=True
                    )
                    w16_sb = spool.tile([128, TT], F16, tag="w16", bufs=2)
                    nc.vector.tensor_copy(w16_sb, w_ps)
                    scr_sb = spool.tile([128, HC, TT], F16, tag="scr", bufs=2)
                    red_sb = mpool.tile([128, HC], F32, tag="red", bufs=2)
                    for c in range(HC):
                        nc.vector.tensor_mul(
                            scr_sb[:, c, :], ctx_t[:, c, :], w16_sb
                        )
                        nc.vector.tensor_reduce(
                            red_sb[:, c:c + 1], scr_sb[:, c, :],
                            axis=mybir.AxisListType.X, op=ALU.add,
                        )
                    nc.vector.tensor_add(u_sb, u_sb, red_sb)
                # normalize + write outputs
                l_sum = mpool.tile([1, 1], F32, tag="lsum", bufs=2)
                nc.vector.tensor_reduce(
                    l_sum, lpart, axis=mybir.AxisListType.X, op=ALU.add
                )
                rcp = mpool.tile([1, 1], F32, tag="rcp", bufs=2)
                nc.vector.reciprocal(rcp, l_sum)
                p_sb = mpool.tile([1, T], F32, tag="p", bufs=2)
                nc.scalar.activation(p_sb, e_sb, AF.Copy, bias=0.0, scale=rcp)
                nc.sync.dma_start(out=p_out[b:b + 1, :], in_=p_sb)
                rcpb_ps = ppool.tile([128, 1], F32, tag="wps", bufs=2)
                nc.tensor.matmul(
                    rcpb_ps, lhsT=ones32_sb, rhs=rcp, start=True, stop=True
                )
                rcpb_sb = mpool.tile([128, 1], F32, tag="rcpb", bufs=2)
                nc.scalar.copy(rcpb_sb, rcpb_ps)
                c_sb = mpool.tile([128, HC], F32, tag="c", bufs=2)
                nc.scalar.activation(c_sb, u_sb, AF.Copy, bias=0.0, scale=rcpb_sb)
                nc.sync.dma_start(
                    out=c_out[b].rearrange("(c p) -> p c", p=128), in_=c_sb
                )

    return nc


def _fix_matmul_waits(nc):
    """Walrus accepts a single sync-wait per instruction on this build; move
    extra waits onto same-engine EventSemaphore ops inserted just before."""
    import concourse.mybir as mybir

    for f in nc.m.functions:
        for bb in f.blocks:
            new = []
            for inst in bb.instructions:
                si = getattr(inst, "sync_info", None)
                if (
                    si is not None
                    and len(si.on_wait) > 1
                    and getattr(inst, "engine", None) is not None
                ):
                    waits = list(si.on_wait)
                    for w in waits[:-1]:
                        new.append(
                            mybir.InstEventSemaphore(
                                name=nc.get_next_instruction_name(),
                                engine=inst.engine,
                                sync_info=mybir.SyncInfo(on_wait=[w], on_update=[]),
                            )
                        )
                    inst.sync_info = mybir.SyncInfo(
                        on_wait=[waits[-1]], on_update=list(si.on_update)
                    )
                new.append(inst)
            bb.instructions[:] = new
    return nc


def _get_nc():
    if "nc" not in _CACHE:
        _CACHE["nc"] = _fix_matmul_waits(_build())
    return _CACHE["nc"]


def _prep_in_maps(x, context, Wq, bq, Wm, bm, v):
    x = np.asarray(x, dtype=np.float32)
    context = np.asarray(context, dtype=np.float32)

    def chunked(a):
        # [H, ...] -> [128, HC, ...] with h = c*128 + p
        return np.ascontiguousarray(
            a.reshape(HC, 128, *a.shape[1:]).transpose(1, 0, *range(2, a.ndim + 1))
        )

    wmh = chunked(np.asarray(Wm, dtype=np.float32).T.astype(np.float16))
    wqh = chunked(np.asarray(Wq, dtype=np.float32).T.astype(np.float16))
    vh = chunked(np.asarray(v, dtype=np.float32).astype(np.float16))
    bqh = chunked(np.asarray(bq, dtype=np.float32))
    bmh = chunked(np.asarray(bm, dtype=np.float32))

    in_maps = []
    for i in range(NCORES):
        sl = slice(i * BL, (i + 1) * BL)
        # context[b].T -> [H, T] -> [128, HC, T] per batch
        ctx = context[sl].transpose(0, 2, 1).astype(np.float16)
        ctxh = np.ascontiguousarray(ctx.reshape(BL, HC, 128, T).transpose(0, 2, 1, 3))
        xhp = chunked(x[sl].T.astype(np.float16))
        in_maps.append(
            {
                "ctxh": ctxh,
                "wmh": wmh,
                "wqh": wqh,
                "xh": xhp,
                "vh": vh,
                "bqh": bqh,
                "bmh": bmh,
            }
        )
    return in_maps


def kernel(x, context, mask, Wq, bq, Wm, bm, v, _trace=False):
    from concourse import bass_utils

    nc = _get_nc()
    in_maps = _prep_in_maps(x, context, Wq, bq, Wm, bm, v)
    res = bass_utils.run_bass_kernel_spmd(
        nc, in_maps, core_ids=list(range(NCORES)), trace=_trace
    )
    if _trace and res.exec_time_ns is not None:
        print(f"HW exec time: {res.exec_time_ns} ns")
    c = np.concatenate([res.results[i]["c_out"] for i in range(NCORES)], axis=0)
    p = np.concatenate([res.results[i]["p_out"] for i in range(NCORES)], axis=0)
    return c, p


# revision 21
# speedup vs baseline: 1.0707x; 1.0623x over previous
"""Bahdanau-attention kernel for Trainium2, data-parallel over batch on 8 cores.

Computation (per batch b):
    q   = x[b] @ Wq.T + bq + bm                    # [H]      (stage A)
    pre = ctx[b] @ Wm.T (+ q broadcast)            # [T, H]   (main matmuls)
    s   = tanh(pre) . v                            # [T]      (v-dot matmuls)
    e   = exp(s);  L = sum(e)                      # softmax without max-sub
    score = e / L
    c   = sum_t e[t] * ctx[b,t,:] / L              # weighted ctx sum (DVE)

Layout: all device inputs are pre-arranged on the host into the SBUF tile
layout [partition, h-chunk, ...] (fp16 on the matmul path) so every DMA is
contiguous per partition. Context streams in [128, HC, 512] tiles over three
DMA queues; the main matmul contracts H on the partition dim at 1 col/cycle
(fp16), exp(s) is broadcast across partitions via a ones-matmul, and the
weighted context sum runs on the vector engine. Dependency-free warm-up
matmuls flip the PE clock gate to full rate during the prologue DMAs.
"""

import numpy as np

B, T, H = 32, 2048, 1024
NCORES = 8
BL = B // NCORES  # batches per core
HC = H // 128     # 128-partition chunks of H
TT = 512          # t-tile (one fp32 PSUM bank)
NTT = T // TT

_CACHE = {}


def _build():
    import concourse.bass as bass
    import concourse.mybir as mybir
    import concourse.tile as tile

    F32 = mybir.dt.float32
    F16 = mybir.dt.float16
    AF = mybir.ActivationFunctionType
    ALU = mybir.AluOpType

    nc = bass.Bass("TRN2", target_bir_lowering=False, debug=False)

    ctxh = nc.dram_tensor("ctxh", [BL, 128, HC, T], F16, kind="ExternalInput").ap()
    wmh = nc.dram_tensor("wmh", [128, HC, H], F16, kind="ExternalInput").ap()
    wqh = nc.dram_tensor("wqh", [128, HC, H], F16, kind="ExternalInput").ap()
    xh = nc.dram_tensor("xh", [128, HC, BL], F16, kind="ExternalInput").ap()
    vh = nc.dram_tensor("vh", [128, HC], F16, kind="ExternalInput").ap()
    bqh = nc.dram_tensor("bqh", [128, HC], F32, kind="ExternalInput").ap()
    bmh = nc.dram_tensor("bmh", [128, HC], F32, kind="ExternalInput").ap()
    c_out = nc.dram_tensor("c_out", [BL, H], F32, kind="ExternalOutput").ap()
    p_out = nc.dram_tensor("p_out", [BL, T], F32, kind="ExternalOutput").ap()

    with tile.TileContext(nc) as tc:
        with (
            tc.tile_pool(name="weights", bufs=1) as wpool,
            tc.tile_pool(name="stream", bufs=3) as spool,
            tc.tile_pool(name="small", bufs=2) as mpool,
            tc.tile_pool(name="psum", bufs=1, space="PSUM") as ppool,
        ):
            # ---- PE warm-up: dependency-free full-K matmuls flip the HAM
            # clock gate to 8/8 while the prologue DMAs are in flight
            junkA_sb = wpool.tile([128, 128], F16)
            nc.vector.memset(junkA_sb, 0.0)
            junkB_sb = wpool.tile([128, TT], F16)
            nc.vector.memset(junkB_sb, 0.0)
            warm_ps = ppool.tile([128, TT], F32, tag="wps", bufs=2, name="warm_ps")
            ones_sb = wpool.tile([1, 128], F16)
            nc.vector.memset(ones_sb, 1.0)
            for _ in range(20):
                nc.tensor.matmul(
                    warm_ps, lhsT=junkA_sb, rhs=junkB_sb, start=True, stop=True
                )

            # ---- resident tensors -------------------------------------
            # batch-0 context chunks on the SWDGE path (they gate the first
            # matmuls); Wm halves take the two HWDGE queues
            ctx00_sb = spool.tile(
                [128, HC, TT], F16, tag="ctx", bufs=3, name="ctx00_sb"
            )
            for c in range(HC):
                nc.gpsimd.dma_start(
                    out=ctx00_sb[:, c, :], in_=ctxh[0, :, c, 0:TT]
                )
            wm_sb = wpool.tile([128, HC, H], F16)  # [p, hchunk, o]
            nc.sync.dma_start(out=wm_sb[:, 0:2, :], in_=wmh[:, 0:2, :])
            nc.scalar.dma_start(out=wm_sb[:, 2:4, :], in_=wmh[:, 2:4, :])
            nc.sync.dma_start(out=wm_sb[:, 4:6, :], in_=wmh[:, 4:6, :])
            nc.scalar.dma_start(out=wm_sb[:, 6:8, :], in_=wmh[:, 6:8, :])
            xT_sb = wpool.tile([128, HC, BL], F16)
            nc.gpsimd.dma_start(out=xT_sb, in_=xh)
            v_sb = wpool.tile([128, HC], F16)
            nc.gpsimd.dma_start(out=v_sb, in_=vh)
            bq_sb = wpool.tile([128, HC], F32)
            nc.scalar.dma_start(out=bq_sb, in_=bqh)
            bm_sb = wpool.tile([128, HC], F32)
            nc.scalar.dma_start(out=bm_sb, in_=bmh)
            bqm_sb = wpool.tile([128, HC], F32)
            nc.vector.tensor_add(bqm_sb, bq_sb, bm_sb)
            wq_sb = wpool.tile([128, HC, H], F16)
            nc.sync.dma_start(out=wq_sb[:, 0:4, :], in_=wqh[:, 0:4, :])
            nc.scalar.dma_start(out=wq_sb[:, 4:8, :], in_=wqh[:, 4:8, :])
            ones32_sb = wpool.tile([1, 128], F32)
            nc.vector.memset(ones32_sb, 1.0)
            onescol_sb = wpool.tile([128, 1], F16)
            nc.vector.memset(onescol_sb, 1.0)
            qb_sb = wpool.tile([128, HC, BL], F32)  # q + bq + bm, [o-part, ot, b]

            # ---- main loop --------------------------------------------
            for b in range(BL):
                e_sb = mpool.tile([1, T], F16, tag="e", bufs=2)
                lpart = mpool.tile([1, NTT], F32, tag="lp", bufs=2)
                u_sb = mpool.tile([128, HC], F32, tag="u", bufs=2)
                nc.vector.memset(u_sb, 0.0)
                for tt in range(NTT):
                    tsl = slice(tt * TT, (tt + 1) * TT)
                    if b == 0 and tt == 0:
                        ctx_t = ctx00_sb
                    else:
                        ctx_t = spool.tile([128, HC, TT], F16, tag="ctx", bufs=3)
                        eng = [nc.sync, nc.scalar, nc.gpsimd][(b * NTT + tt) % 3]
                        eng.dma_start(out=ctx_t, in_=ctxh[b, :, :, tsl])
                    s_ps = ppool.tile([1, TT], F32, tag="sps", bufs=3)
                    s4_ps = ppool.tile([128, TT], F32, tag="wps", bufs=2, name="s4_ps")
                    nc.vector.memset(s4_ps, 0.0)
                    th_tiles = {}

                    def tile_epilogue(ot, hc_ps):
                        if b == 0 and tt == 0:
                            # stage A interleaved with the first tile's mains
                            q_ps = ppool.tile(
                                [128, BL], F32, tag="sps", bufs=3, name="q_ps"
                            )
                            for c in range(HC):
                                nc.tensor.matmul(
                                    q_ps,
                                    lhsT=wq_sb[:, c, ot * 128:(ot + 1) * 128],
                                    rhs=xT_sb[:, c, :],
                                    start=(c == 0),
                                    stop=(c == HC - 1),
                                )
                            nc.scalar.activation(
                                qb_sb[:, ot, :], q_ps, AF.Identity,
                                bias=bqm_sb[:, ot:ot + 1], scale=1.0,
                            )
                        th_sb = spool.tile([128, TT], F16, tag="tanh", bufs=10)
                        nc.scalar.activation(
                            th_sb, hc_ps, AF.Tanh,
                            bias=qb_sb[:, ot, b:b + 1], scale=1.0,
                        )
                        th_tiles[ot] = th_sb

                    if b == 0 and tt == 0:
                        # c-outer in groups of <=3 o-tiles so the first mains
                        # consume Wm/ctx chunks at their DMA arrival rate
                        for grp in ((0, 1, 2), (3, 4, 5), (6, 7)):
                            tiles = {
                                ot: ppool.tile(
                                    [128, TT], F32, tag="hcps", bufs=3,
                                    name=f"hc_ps_g{ot}",
                                )
                                for ot in grp
                            }
                            for c in range(HC):
                                for ot in grp:
                                    nc.tensor.matmul(
                                        tiles[ot],
                                        lhsT=wm_sb[:, c, ot * 128:(ot + 1) * 128],
                                        rhs=ctx_t[:, c, :],
                                        start=(c == 0),
                                        stop=(c == HC - 1),
                                    )
                            for ot in grp:
                                tile_epilogue(ot, tiles[ot])
                    else:
                        for ot in range(HC):
                            hc_ps = ppool.tile([128, TT], F32, tag="hcps", bufs=3)
                            for c in range(HC):
                                nc.tensor.matmul(
                                    hc_ps,
                                    lhsT=wm_sb[:, c, ot * 128:(ot + 1) * 128],
                                    rhs=ctx_t[:, c, :],
                                    start=(c == 0),
                                    stop=(c == HC - 1),
                                )
                            tile_epilogue(ot, hc_ps)

                    for half in (0, 1):
                        for ot in range(half * 4, half * 4 + 4):
                            j = 32 * (ot % 4)
                            nc.tensor.matmul(
                                s4_ps[j:j + 1, :],
                                lhsT=v_sb[:, ot:ot + 1],
                                rhs=th_tiles[ot],
                                start=(half == 0),
                                stop=(half == 1),
                                tile_position=(0, j),
                                skip_group_check=True,
                            )
                    s4_sb = spool.tile([128, TT], F16, tag="s4", bufs=2)
                    nc.scalar.copy(s4_sb, s4_ps)
                    nc.tensor.matmul(
                        s_ps, lhsT=onescol_sb, rhs=s4_sb, start=True, stop=True
                    )
                    e_slice = e_sb[:, tsl]
                    nc.scalar.activation(
                        e_slice, s_ps, AF.Exp,
                        scale=1.0, accum_out=lpart[:, tt:tt + 1],
                    )
                    w_ps = ppool.tile([128, TT], F32, tag="wps", bufs=2)
                    nc.tensor.matmul(
                        w_ps, lhsT=ones_sb, rhs=e_slice, start=True, stop=True
                    )
                    w16_sb = spool.tile([128, TT], F16, tag="w16", bufs=2)
                    nc.vector.tensor_copy(w16_sb, w_ps)
                    scr_sb = spool.tile([128, HC, TT], F16, tag="scr", bufs=2)
                    red_sb = mpool.tile([128, HC], F32, tag="red", bufs=2)
                    for c in range(HC):
                        nc.vector.tensor_mul(
                            scr_sb[:, c, :], ctx_t[:, c, :], w16_sb
                        )
                        nc.vector.tensor_reduce(
                            red_sb[:, c:c + 1], scr_sb[:, c, :],
                            axis=mybir.AxisListType.X, op=ALU.add,
                        )
                    nc.vector.tensor_add(u_sb, u_sb, red_sb)
                # normalize + write outputs
                l_sum = mpool.tile([1, 1], F32, tag="lsum", bufs=2)
                nc.vector.tensor_reduce(
                    l_sum, lpart, axis=mybir.AxisListType.X, op=ALU.add
                )
                rcp = mpool.tile([1, 1], F32, tag="rcp", bufs=2)
                nc.vector.reciprocal(rcp, l_sum)
                p_sb = mpool.tile([1, T], F32, tag="p", bufs=2)
                nc.scalar.activation(p_sb, e_sb, AF.Copy, bias=0.0, scale=rcp)
                nc.sync.dma_start(out=p_out[b:b + 1, :], in_=p_sb)
                rcpb_ps = ppool.tile([128, 1], F32, tag="wps", bufs=2)
                nc.tensor.matmul(
                    rcpb_ps, lhsT=ones32_sb, rhs=rcp, start=True, stop=True
                )
                rcpb_sb = mpool.tile([128, 1], F32, tag="rcpb", bufs=2)
                nc.scalar.copy(rcpb_sb, rcpb_ps)
                c_sb = mpool.tile([128, HC], F32, tag="c", bufs=2)
                nc.scalar.activation(c_sb, u_sb, AF.Copy, bias=0.0, scale=rcpb_sb)
                nc.sync.dma_start(
                    out=c_out[b].rearrange("(c p) -> p c", p=128), in_=c_sb
                )

    return nc


def _fix_matmul_waits(nc):
    """Walrus accepts a single sync-wait per instruction on this build; move
    extra waits onto same-engine EventSemaphore ops inserted just before."""
    import concourse.mybir as mybir

    for f in nc.m.functions:
        for bb in f.blocks:
            new = []
            for inst in bb.instructions:
                si = getattr(inst, "sync_info", None)
                if (
                    si is not None
                    and len(si.on_wait) > 1
                    and getattr(inst, "engine", None) is not None
                ):
                    waits = list(si.on_wait)
                    for w in waits[:-1]:
                        new.append(
                            mybir.InstEventSemaphore(
                                name=nc.get_next_instruction_name(),
                                engine=inst.engine,
                                sync_info=mybir.SyncInfo(on_wait=[w], on_update=[]),
                            )
                        )
                    inst.sync_info = mybir.SyncInfo(
                        on_wait=[waits[-1]], on_update=list(si.on_update)
                    )
                new.append(inst)
            bb.instructions[:] = new
    return nc


def _get_nc():
    if "nc" not in _CACHE:
        _CACHE["nc"] = _fix_matmul_waits(_build())
    return _CACHE["nc"]


def _prep_in_maps(x, context, Wq, bq, Wm, bm, v):
    x = np.asarray(x, dtype=np.float32)
    context = np.asarray(context, dtype=np.float32)

    def chunked(a):
        # [H, ...] -> [128, HC, ...] with h = c*128 + p
        return np.ascontiguousarray(
            a.reshape(HC, 128, *a.shape[1:]).transpose(1, 0, *range(2, a.ndim + 1))
        )

    wmh = chunked(np.asarray(Wm, dtype=np.float32).T.astype(np.float16))
    wqh = chunked(np.asarray(Wq, dtype=np.float32).T.astype(np.float16))
    vh = chunked(np.asarray(v, dtype=np.float32).astype(np.float16))
    bqh = chunked(np.asarray(bq, dtype=np.float32))
    bmh = chunked(np.asarray(bm, dtype=np.float32))

    in_maps = []
    for i in range(NCORES):
        sl = slice(i * BL, (i + 1) * BL)
        # context[b].T -> [H, T] -> [128, HC, T] per batch
        ctx = context[sl].transpose(0, 2, 1).astype(np.float16)
        ctxh = np.ascontiguousarray(ctx.reshape(BL, HC, 128, T).transpose(0, 2, 1, 3))
        xhp = chunked(x[sl].T.astype(np.float16))
        in_maps.append(
            {
                "ctxh": ctxh,
                "wmh": wmh,
                "wqh": wqh,
                "xh": xhp,
                "vh": vh,
                "bqh": bqh,
                "bmh": bmh,
            }
        )
    return in_maps


def kernel(x, context, mask, Wq, bq, Wm, bm, v, _trace=False):
    from concourse import bass_utils

    nc = _get_nc()
    in_maps = _prep_in_maps(x, context, Wq, bq, Wm, bm, v)
    res = bass_utils.run_bass_kernel_spmd(
        nc, in_maps, core_ids=list(range(NCORES)), trace=_trace
    )
    if _trace and res.exec_time_ns is not None:
        print(f"HW exec time: {res.exec_time_ns} ns")
    c = np.concatenate([res.results[i]["c_out"] for i in range(NCORES)], axis=0)
    p = np.concatenate([res.results[i]["p_out"] for i in range(NCORES)], axis=0)
    return c, p


# revision 22
# speedup vs baseline: 1.1039x; 1.0310x over previous
"""Bahdanau-attention kernel for Trainium2, data-parallel over batch on 8 cores.

Computation (per batch b):
    q   = x[b] @ Wq.T + bq + bm                    # [H]      (stage A)
    pre = ctx[b] @ Wm.T (+ q broadcast)            # [T, H]   (main matmuls)
    s   = tanh(pre) . v                            # [T]      (v-dot matmuls)
    e   = exp(s);  L = sum(e)                      # softmax without max-sub
    score = e / L
    c   = sum_t e[t] * ctx[b,t,:] / L              # weighted ctx sum (DVE)

Layout: all device inputs are pre-arranged on the host into the SBUF tile
layout [partition, h-chunk, ...] (fp16 on the matmul path) so every DMA is
contiguous per partition. Context streams in [128, HC, 512] tiles over three
DMA queues; the main matmul contracts H on the partition dim at 1 col/cycle
(fp16), exp(s) is broadcast across partitions via a ones-matmul, and the
weighted context sum runs on the vector engine. Dependency-free warm-up
matmuls flip the PE clock gate to full rate during the prologue DMAs.
"""

import numpy as np

B, T, H = 32, 2048, 1024
NCORES = 8
BL = B // NCORES  # batches per core
HC = H // 128     # 128-partition chunks of H
TT = 512          # t-tile (one fp32 PSUM bank)
NTT = T // TT

_CACHE = {}


def _build():
    import concourse.bass as bass
    import concourse.mybir as mybir
    import concourse.tile as tile

    F32 = mybir.dt.float32
    F16 = mybir.dt.float16
    AF = mybir.ActivationFunctionType
    ALU = mybir.AluOpType

    nc = bass.Bass("TRN2", target_bir_lowering=False, debug=False)

    ctxh = nc.dram_tensor("ctxh", [BL, 128, HC, T], F16, kind="ExternalInput").ap()
    wmh = nc.dram_tensor("wmh", [128, HC, H], F16, kind="ExternalInput").ap()
    wqh = nc.dram_tensor("wqh", [128, HC, H], F16, kind="ExternalInput").ap()
    xh = nc.dram_tensor("xh", [128, HC, BL], F16, kind="ExternalInput").ap()
    vh = nc.dram_tensor("vh", [128, HC], F16, kind="ExternalInput").ap()
    bqh = nc.dram_tensor("bqh", [128, HC], F32, kind="ExternalInput").ap()
    bmh = nc.dram_tensor("bmh", [128, HC], F32, kind="ExternalInput").ap()
    c_out = nc.dram_tensor("c_out", [BL, H], F32, kind="ExternalOutput").ap()
    p_out = nc.dram_tensor("p_out", [BL, T], F32, kind="ExternalOutput").ap()

    with tile.TileContext(nc) as tc:
        with (
            tc.tile_pool(name="weights", bufs=1) as wpool,
            tc.tile_pool(name="stream", bufs=3) as spool,
            tc.tile_pool(name="small", bufs=2) as mpool,
            tc.tile_pool(name="psum", bufs=1, space="PSUM") as ppool,
        ):
            # ---- PE warm-up: dependency-free full-K matmuls flip the HAM
            # clock gate to 8/8 while the prologue DMAs are in flight
            junkA_sb = wpool.tile([128, 128], F16)
            nc.vector.memset(junkA_sb, 0.0)
            junkB_sb = wpool.tile([128, TT], F16)
            nc.vector.memset(junkB_sb, 0.0)
            warm_ps = ppool.tile([128, TT], F32, tag="wps", bufs=2, name="warm_ps")
            ones_sb = wpool.tile([1, 128], F16)
            nc.vector.memset(ones_sb, 1.0)
            for _ in range(20):
                nc.tensor.matmul(
                    warm_ps, lhsT=junkA_sb, rhs=junkB_sb, start=True, stop=True
                )

            # ---- resident tensors -------------------------------------
            # batch-0 context chunks on the SWDGE path (they gate the first
            # matmuls); Wm halves take the two HWDGE queues
            ctx00_sb = spool.tile(
                [128, HC, TT], F16, tag="ctx", bufs=3, name="ctx00_sb"
            )
            for c in range(HC):
                nc.gpsimd.dma_start(
                    out=ctx00_sb[:, c, :], in_=ctxh[0, :, c, 0:TT]
                )
            wm_sb = wpool.tile([128, HC, H], F16)  # [p, hchunk, o]
            nc.sync.dma_start(out=wm_sb[:, 0:2, :], in_=wmh[:, 0:2, :])
            nc.scalar.dma_start(out=wm_sb[:, 2:4, :], in_=wmh[:, 2:4, :])
            nc.sync.dma_start(out=wm_sb[:, 4:6, :], in_=wmh[:, 4:6, :])
            nc.scalar.dma_start(out=wm_sb[:, 6:8, :], in_=wmh[:, 6:8, :])
            xT_sb = wpool.tile([128, HC, BL], F16)
            nc.gpsimd.dma_start(out=xT_sb, in_=xh)
            v_sb = wpool.tile([128, HC], F16)
            nc.gpsimd.dma_start(out=v_sb, in_=vh)
            bq_sb = wpool.tile([128, HC], F32)
            nc.scalar.dma_start(out=bq_sb, in_=bqh)
            bm_sb = wpool.tile([128, HC], F32)
            nc.scalar.dma_start(out=bm_sb, in_=bmh)
            bqm_sb = wpool.tile([128, HC], F32)
            nc.vector.tensor_add(bqm_sb, bq_sb, bm_sb)
            wq_sb = wpool.tile([128, HC, H], F16)
            nc.sync.dma_start(out=wq_sb[:, 0:4, :], in_=wqh[:, 0:4, :])
            nc.scalar.dma_start(out=wq_sb[:, 4:8, :], in_=wqh[:, 4:8, :])
            ones32_sb = wpool.tile([1, 128], F32)
            nc.vector.memset(ones32_sb, 1.0)
            onescol_sb = wpool.tile([128, 1], F16)
            nc.vector.memset(onescol_sb, 1.0)
            qb_sb = wpool.tile([128, HC, BL], F32)  # q + bq + bm, [o-part, ot, b]

            # ---- main loop --------------------------------------------
            for b in range(BL):
                e_sb = mpool.tile([1, T], F16, tag="e", bufs=2)
                lpart = mpool.tile([1, NTT], F32, tag="lp", bufs=2)
                u_sb = mpool.tile([128, HC], F32, tag="u", bufs=2)
                nc.vector.memset(u_sb, 0.0)
                for tt in range(NTT):
                    tsl = slice(tt * TT, (tt + 1) * TT)
                    if b == 0 and tt == 0:
                        ctx_t = ctx00_sb
                    else:
                        ctx_t = spool.tile([128, HC, TT], F16, tag="ctx", bufs=3)
                        eng = [nc.sync, nc.scalar, nc.gpsimd][(b * NTT + tt) % 3]
                        eng.dma_start(out=ctx_t, in_=ctxh[b, :, :, tsl])
                    s_ps = ppool.tile([1, TT], F32, tag="sps", bufs=2)
                    s4_ps = ppool.tile([128, TT], F32, tag="wps", bufs=2, name="s4_ps")
                    nc.vector.memset(s4_ps, 0.0)
                    th_tiles = {}

                    def tile_epilogue(ot, hc_ps):
                        if b == 0 and tt == 0:
                            # stage A interleaved with the first tile's mains
                            q_ps = ppool.tile(
                                [128, BL], F32, tag="sps", bufs=2, name="q_ps"
                            )
                            for c in range(HC):
                                nc.tensor.matmul(
                                    q_ps,
                                    lhsT=wq_sb[:, c, ot * 128:(ot + 1) * 128],
                                    rhs=xT_sb[:, c, :],
                                    start=(c == 0),
                                    stop=(c == HC - 1),
                                )
                            nc.scalar.activation(
                                qb_sb[:, ot, :], q_ps, AF.Identity,
                                bias=bqm_sb[:, ot:ot + 1], scale=1.0,
                            )
                        th_sb = spool.tile([128, TT], F16, tag="tanh", bufs=10)
                        nc.scalar.activation(
                            th_sb, hc_ps, AF.Tanh,
                            bias=qb_sb[:, ot, b:b + 1], scale=1.0,
                        )
                        th_tiles[ot] = th_sb

                    if b == 0 and tt == 0:
                        # c-outer in groups of <=3 o-tiles so the first mains
                        # consume Wm/ctx chunks at their DMA arrival rate
                        for grp in ((0, 1, 2), (3, 4, 5), (6, 7)):
                            tiles = {
                                ot: ppool.tile(
                                    [128, TT], F32, tag="hcps", bufs=4,
                                    name=f"hc_ps_g{ot}",
                                )
                                for ot in grp
                            }
                            for c in range(HC):
                                for ot in grp:
                                    nc.tensor.matmul(
                                        tiles[ot],
                                        lhsT=wm_sb[:, c, ot * 128:(ot + 1) * 128],
                                        rhs=ctx_t[:, c, :],
                                        start=(c == 0),
                                        stop=(c == HC - 1),
                                    )
                            for ot in grp:
                                tile_epilogue(ot, tiles[ot])
                    else:
                        for ot in range(HC):
                            hc_ps = ppool.tile([128, TT], F32, tag="hcps", bufs=4)
                            for c in range(HC):
                                nc.tensor.matmul(
                                    hc_ps,
                                    lhsT=wm_sb[:, c, ot * 128:(ot + 1) * 128],
                                    rhs=ctx_t[:, c, :],
                                    start=(c == 0),
                                    stop=(c == HC - 1),
                                )
                            tile_epilogue(ot, hc_ps)

                    for half in (0, 1):
                        for ot in range(half * 4, half * 4 + 4):
                            j = 32 * (ot % 4)
                            nc.tensor.matmul(
                                s4_ps[j:j + 1, :],
                                lhsT=v_sb[:, ot:ot + 1],
                                rhs=th_tiles[ot],
                                start=(half == 0),
                                stop=(half == 1),
                                tile_position=(0, j),
                                skip_group_check=True,
                            )
                    s4_sb = spool.tile([128, TT], F16, tag="s4", bufs=2)
                    nc.scalar.copy(s4_sb, s4_ps)
                    nc.tensor.matmul(
                        s_ps, lhsT=onescol_sb, rhs=s4_sb, start=True, stop=True
                    )
                    e_slice = e_sb[:, tsl]
                    nc.scalar.activation(
                        e_slice, s_ps, AF.Exp,
                        scale=1.0, accum_out=lpart[:, tt:tt + 1],
                    )
                    w_ps = ppool.tile([128, TT], F32, tag="wps", bufs=2)
                    nc.tensor.matmul(
                        w_ps, lhsT=ones_sb, rhs=e_slice, start=True, stop=True
                    )
                    w16_sb = spool.tile([128, TT], F16, tag="w16", bufs=2)
                    nc.vector.tensor_copy(w16_sb, w_ps)
                    scr_sb = spool.tile([128, HC, TT], F16, tag="scr", bufs=2)
                    red_sb = mpool.tile([128, HC], F32, tag="red", bufs=2)
                    for c in range(HC):
                        nc.vector.tensor_mul(
                            scr_sb[:, c, :], ctx_t[:, c, :], w16_sb
                        )
                        nc.vector.tensor_reduce(
                            red_sb[:, c:c + 1], scr_sb[:, c, :],
                            axis=mybir.AxisListType.X, op=ALU.add,
                        )
                    nc.vector.tensor_add(u_sb, u_sb, red_sb)
                # normalize + write outputs
                l_sum = mpool.tile([1, 1], F32, tag="lsum", bufs=2)
                nc.vector.tensor_reduce(
                    l_sum, lpart, axis=mybir.AxisListType.X, op=ALU.add
                )
                rcp = mpool.tile([1, 1], F32, tag="rcp", bufs=2)
                nc.vector.reciprocal(rcp, l_sum)
                p_sb = mpool.tile([1, T], F32, tag="p", bufs=2)
                nc.scalar.activation(p_sb, e_sb, AF.Copy, bias=0.0, scale=rcp)
                nc.sync.dma_start(out=p_out[b:b + 1, :], in_=p_sb)
                rcpb_ps = ppool.tile([128, 1], F32, tag="wps", bufs=2)
                nc.tensor.matmul(
                    rcpb_ps, lhsT=ones32_sb, rhs=rcp, start=True, stop=True
                )
                rcpb_sb = mpool.tile([128, 1], F32, tag="rcpb", bufs=2)
                nc.scalar.copy(rcpb_sb, rcpb_ps)
                c_sb = mpool.tile([128, HC], F32, tag="c", bufs=2)
                nc.scalar.activation(c_sb, u_sb, AF.Copy, bias=0.0, scale=rcpb_sb)
                nc.sync.dma_start(
                    out=c_out[b].rearrange("(c p) -> p c", p=128), in_=c_sb
                )

    return nc


def _fix_matmul_waits(nc):
    """Walrus accepts a single sync-wait per instruction on this build; move
    extra waits onto same-engine EventSemaphore ops inserted just before."""
    import concourse.mybir as mybir

    for f in nc.m.functions:
        for bb in f.blocks:
            new = []
            for inst in bb.instructions:
                si = getattr(inst, "sync_info", None)
                if (
                    si is not None
                    and len(si.on_wait) > 1
                    and getattr(inst, "engine", None) is not None
                ):
                    waits = list(si.on_wait)
                    for w in waits[:-1]:
                        new.append(
                            mybir.InstEventSemaphore(
                                name=nc.get_next_instruction_name(),
                                engine=inst.engine,
                                sync_info=mybir.SyncInfo(on_wait=[w], on_update=[]),
                            )
                        )
                    inst.sync_info = mybir.SyncInfo(
                        on_wait=[waits[-1]], on_update=list(si.on_update)
                    )
                new.append(inst)
            bb.instructions[:] = new
    return nc


def _get_nc():
    if "nc" not in _CACHE:
        _CACHE["nc"] = _fix_matmul_waits(_build())
    return _CACHE["nc"]


def _prep_in_maps(x, context, Wq, bq, Wm, bm, v):
    x = np.asarray(x, dtype=np.float32)
    context = np.asarray(context, dtype=np.float32)

    def chunked(a):
        # [H, ...] -> [128, HC, ...] with h = c*128 + p
        return np.ascontiguousarray(
            a.reshape(HC, 128, *a.shape[1:]).transpose(1, 0, *range(2, a.ndim + 1))
        )

    wmh = chunked(np.asarray(Wm, dtype=np.float32).T.astype(np.float16))
    wqh = chunked(np.asarray(Wq, dtype=np.float32).T.astype(np.float16))
    vh = chunked(np.asarray(v, dtype=np.float32).astype(np.float16))
    bqh = chunked(np.asarray(bq, dtype=np.float32))
    bmh = chunked(np.asarray(bm, dtype=np.float32))

    in_maps = []
    for i in range(NCORES):
        sl = slice(i * BL, (i + 1) * BL)
        # context[b].T -> [H, T] -> [128, HC, T] per batch
        ctx = context[sl].transpose(0, 2, 1).astype(np.float16)
        ctxh = np.ascontiguousarray(ctx.reshape(BL, HC, 128, T).transpose(0, 2, 1, 3))
        xhp = chunked(x[sl].T.astype(np.float16))
        in_maps.append(
            {
                "ctxh": ctxh,
                "wmh": wmh,
                "wqh": wqh,
                "xh": xhp,
                "vh": vh,
                "bqh": bqh,
                "bmh": bmh,
            }
        )
    return in_maps


def kernel(x, context, mask, Wq, bq, Wm, bm, v, _trace=False):
    from concourse import bass_utils

    nc = _get_nc()
    in_maps = _prep_in_maps(x, context, Wq, bq, Wm, bm, v)
    res = bass_utils.run_bass_kernel_spmd(
        nc, in_maps, core_ids=list(range(NCORES)), trace=_trace
    )
    if _trace and res.exec_time_ns is not None:
        print(f"HW exec time: {res.exec_time_ns} ns")
    c = np.concatenate([res.results[i]["c_out"] for i in range(NCORES)], axis=0)
    p = np.concatenate([res.results[i]["p_out"] for i in range(NCORES)], axis=0)
    return c, p
